# revision 22
# baseline (speedup 1.0000x reference)
"""Trainium2 Bass kernel for nn_BitwiseHashing.

Computes out = tanh(mean_l(x) @ W.T + b) for x:[12,8192,1024] f32,
W:[64,1024], b:[64] -> out:[8192,64].

Strategy (data-parallel over 8 NeuronCores):
  - shard x along batch dim: 1024 rows per core (48 MiB each, streamed).
  - host pre-transposes W to wt = bf16(W.T / L) [1024,64]; bias as bf16 [1,64].
  - x is streamed in 1 MiB DMAs: each [128,2048] f32 tile holds a SUPERBLOCK
    of 256 contiguous batch rows, two per partition (partition p carries rows
    2p and 2p+1 of the superblock: cols 0:1024 = even row, 1024:2048 = odd).
    Halving the DMA count (vs 512 KiB tiles) halves the trigger+semaphore
    load on the two HWDGE ring engines and the DVE instruction overhead.
  - per superblock: 12 L-slices on the two HWDGE rings, DVE add tree casting
    f32 -> bf16 at first touch (later adds at 2x DVE rate), PE-transpose the
    16 [128,128] d-chunks of the bf16 sum (bf16 transposes ~4x cheaper than
    f32), PSUM->SBUF chunk copies on DVE, two bf16 matmul groups (one per
    row-parity half) accumulating in f32 PSUM [128,64] with bias pre-loaded
    via a C=1 ones-matmul, tanh on ScalarE. The row-parity interleave in y
    is undone on the host with a cheap transpose-reshape.
  - software pipeline keeps both HWDGE rings fed with nothing but x-load
    triggers: the previous superblock's copies/matmuls/tanh are emitted
    mid-way through the next superblock's add sequence (their deps are
    complete by then), and y writes sit behind the next superblock's
    triggers in the sync FIFO. The final superblock is processed per
    row-parity half to overlap its DVE tail with PE transposes.
"""

import numpy as np

import concourse.bacc as bacc
import concourse.mybir as mybir
from concourse import tile
from concourse.masks import make_identity
from concourse.bass_utils import run_bass_kernel_spmd

L, B, D, K = 12, 8192, 1024, 64
NCORES = 8
BS = B // NCORES      # 1024 batch rows per core
P = 128               # partitions
R = 2                 # batch rows per partition per tile
D2 = R * D            # tile free width (2048)
NSB = BS // (R * P)   # 4 superblocks of 256 rows per core
NDC = D // P          # 8 contraction chunks per row-parity half
F32 = mybir.dt.float32
BF16 = mybir.dt.bfloat16

_nc_cache = None


def _build():
    global _nc_cache
    if _nc_cache is not None:
        return _nc_cache

    nc = bacc.Bacc("TRN2", target_bir_lowering=False, debug=False)
    # x viewed as row-pairs: x[l, rp, :] = orig rows (2rp, 2rp+1) concatenated
    x = nc.dram_tensor("x", [L, BS // R, D2], F32, kind="ExternalInput")
    wt = nc.dram_tensor("wt", [D, K], BF16, kind="ExternalInput")
    bias = nc.dram_tensor("bias", [1, K], BF16, kind="ExternalInput")
    # y[sb, j, p, :] = output row sb*256 + 2p + j (host untangles)
    y = nc.dram_tensor("y", [NSB, R, P, K], F32, kind="ExternalOutput")

    with tile.TileContext(nc) as tc:
        with (
            tc.tile_pool(name="const", bufs=1) as cpool,
            tc.tile_pool(name="xin", bufs=17) as xpool,
            tc.tile_pool(name="part", bufs=11) as ppool,
            tc.tile_pool(name="xt", bufs=2) as tpool,
            tc.tile_pool(name="out", bufs=4) as opool,
            tc.tile_pool(name="pt", bufs=2, space="PSUM") as pt_pool,
            tc.tile_pool(name="po", bufs=2, space="PSUM") as po_pool,
        ):
            # constants go over the SWDGE queue to keep both HWDGE rings
            # free for the x stream from t=0
            wt_sb = cpool.tile([P, NDC * K], BF16)
            for dc in range(NDC):
                nc.gpsimd.dma_start(
                    out=wt_sb[:, dc * K:(dc + 1) * K],
                    in_=wt.ap()[dc * P:(dc + 1) * P, :],
                )
            bias_sb = cpool.tile([1, K], BF16)
            nc.gpsimd.dma_start(out=bias_sb[:], in_=bias.ap())
            ones_sb = cpool.tile([1, P], BF16)
            nc.gpsimd.memset(ones_sb[:], 1.0)
            ident = cpool.tile([P, P], BF16)
            make_identity(nc, ident[:])

            xap = x.ap()
            yap = y.ap()

            def issue_loads(sb):
                r0 = sb * P
                xt = []
                for l in range(L):
                    xl = xpool.tile([P, D2], F32)
                    eng = nc.sync if l % 2 == 0 else nc.scalar
                    eng.dma_start(out=xl[:], in_=xap[l, r0:r0 + P, :])
                    xt.append(xl)
                return xt

            def reduce(xt, mid_cb=None):
                # pair adds WITHIN each DMA ring (even tiles on the sync
                # ring, odd on the scalar ring complete in FIFO order within
                # their ring, so neither chain waits on cross-ring skew),
                # casting f32 -> bf16 at first touch so merges run at 2x DVE
                # rate. mid_cb emits the PREVIOUS superblock's PSUM->SBUF
                # copies (on DVE) + matmuls here, after 4 adds: by then the
                # previous superblock's transposes are long done, so the
                # copies never stall the DVE instruction stream.
                def add(i0, i1):
                    t = ppool.tile([P, D2], BF16)
                    nc.vector.tensor_add(out=t[:], in0=i0[:], in1=i1[:])
                    return t

                ae = add(xt[0], xt[2])
                ao = add(xt[1], xt[3])
                be = add(xt[4], xt[6])
                bo = add(xt[5], xt[7])
                if mid_cb is not None:
                    mid_cb()
                ce = add(ae, be)
                co = add(ao, bo)
                ee = add(xt[8], xt[10])
                eo = add(xt[9], xt[11])
                fe = add(ce, ee)
                fo = add(co, eo)
                return add(fe, fo)

            def transpose_block(acc):
                # transpose the bf16 superblock sum into PSUM
                pt_all = pt_pool.tile([P, D2], BF16)
                for c in range(R * NDC):
                    nc.tensor.transpose(
                        pt_all[:, c * P:(c + 1) * P],
                        acc[:, c * P:(c + 1) * P],
                        ident[:],
                    )
                return pt_all

            def project_core(pt_all):
                # PSUM->SBUF chunk copies on DVE (cheap there, and emitted
                # mid-way through the NEXT superblock's adds so the
                # transposes they wait on are already done), then one
                # K-projection matmul group per row-parity half on PE and
                # tanh on ACT. y writes are emitted separately so they sit
                # BEHIND the next superblock's load triggers in the sync
                # ring FIFO (never head-blocking the x stream).
                xt_all = tpool.tile([P, D2], BF16)
                for c in range(R * NDC):
                    nc.vector.tensor_copy(
                        out=xt_all[:, c * P:(c + 1) * P],
                        in_=pt_all[:, c * P:(c + 1) * P],
                    )

                ots = []
                for j in range(R):
                    po = po_pool.tile([P, K], F32)
                    # bias broadcast: ones[1,128].T @ bias[1,64]
                    nc.tensor.matmul(
                        po[:], lhsT=ones_sb[:], rhs=bias_sb[:],
                        start=True, stop=False,
                    )
                    for dc in range(NDC):
                        c = j * NDC + dc
                        nc.tensor.matmul(
                            po[:],
                            lhsT=xt_all[:, c * P:(c + 1) * P],
                            rhs=wt_sb[:, dc * K:(dc + 1) * K],
                            start=False,
                            stop=(dc == NDC - 1),
                        )
                    ot = opool.tile([P, K], F32)
                    nc.scalar.activation(
                        ot[:], po[:], mybir.ActivationFunctionType.Tanh
                    )
                    ots.append(ot)
                return ots

            def write_y(sb, ots):
                for j in range(R):
                    nc.sync.dma_start(out=yap[sb, j], in_=ots[j][:])

            # --- narrow (128-row, [P,D]-tile) path for the final 256 rows:
            # the post-arrival serial chain scales with tile width, so the
            # kernel TAIL is ~8us shorter when the last rows are processed
            # as two 128-row blocks instead of one 256-row superblock.
            def issue_loads_blk(b):
                rp0 = (NSB - 1) * P + b * (P // R)
                xt = []
                for l in range(L):
                    xl = xpool.tile([P, D], F32)
                    eng = nc.sync if l % 2 == 0 else nc.scalar
                    eng.dma_start(
                        out=xl[:], in_=xap[l, rp0:rp0 + P // R, :]
                    )
                    xt.append(xl)
                return xt

            def reduce1(xt, mid_cb=None):
                def add(i0, i1):
                    t = ppool.tile([P, D], BF16)
                    nc.vector.tensor_add(out=t[:], in0=i0[:], in1=i1[:])
                    return t

                ae = add(xt[0], xt[2])
                ao = add(xt[1], xt[3])
                be = add(xt[4], xt[6])
                bo = add(xt[5], xt[7])
                if mid_cb is not None:
                    mid_cb()
                ce = add(ae, be)
                co = add(ao, bo)
                ee = add(xt[8], xt[10])
                eo = add(xt[9], xt[11])
                fe = add(ce, ee)
                fo = add(co, eo)
                return add(fe, fo)

            def transpose1(acc):
                pt_all = pt_pool.tile([P, D], BF16)
                for dc in range(NDC):
                    nc.tensor.transpose(
                        pt_all[:, dc * P:(dc + 1) * P],
                        acc[:, dc * P:(dc + 1) * P],
                        ident[:],
                    )
                return pt_all

            def project1(pt_all):
                xt_all = tpool.tile([P, D], BF16)
                for dc in range(NDC):
                    nc.vector.tensor_copy(
                        out=xt_all[:, dc * P:(dc + 1) * P],
                        in_=pt_all[:, dc * P:(dc + 1) * P],
                    )
                po = po_pool.tile([P, K], F32)
                nc.tensor.matmul(
                    po[:], lhsT=ones_sb[:], rhs=bias_sb[:],
                    start=True, stop=False,
                )
                for dc in range(NDC):
                    nc.tensor.matmul(
                        po[:],
                        lhsT=xt_all[:, dc * P:(dc + 1) * P],
                        rhs=wt_sb[:, dc * K:(dc + 1) * K],
                        start=False,
                        stop=(dc == NDC - 1),
                    )
                ot = opool.tile([P, K], F32)
                nc.scalar.activation(
                    ot[:], po[:], mybir.ActivationFunctionType.Tanh
                )
                return ot

            def tail1(xt, mid_cb):
                # final 128-row block: the tree is ordered so the two LAST
                # tiles to arrive (l=10 on sync, l=11 on scalar) enter only
                # the final add -- everything else completes while the block
                # is still streaming in. Post-arrival DVE work is just
                # (x10+x11) + one merge per column half, overlapped with PE
                # transposes on the other half.
                def addf(i0, i1):
                    t = ppool.tile([P, D], BF16)
                    nc.vector.tensor_add(out=t[:], in0=i0[:], in1=i1[:])
                    return t

                p0 = addf(xt[0], xt[2])
                p1 = addf(xt[1], xt[3])
                p2 = addf(xt[4], xt[6])
                p3 = addf(xt[5], xt[7])
                mid_cb()
                p4 = addf(xt[8], xt[9])
                m0 = addf(p0, p1)
                m1 = addf(p2, p3)
                m2 = addf(m0, m1)
                m3 = addf(m2, p4)

                H = D // 2
                NH = H // P
                pt_all = pt_pool.tile([P, D], BF16)
                xt_all = tpool.tile([P, D], BF16)
                po = po_pool.tile([P, K], F32)
                for h in range(2):
                    sl = slice(h * H, (h + 1) * H)

                    def addh(a0, s0, a1, s1):
                        t = ppool.tile([P, H], BF16)
                        nc.vector.tensor_add(out=t[:], in0=a0[s0], in1=a1[s1])
                        return t

                    cf = (slice(None), sl)
                    hf = (slice(None), slice(None))
                    lh = addh(xt[10], cf, xt[11], cf)
                    ah = addh(m3, cf, lh, hf)
                    for dcl in range(NH):
                        dc = h * NH + dcl
                        nc.tensor.transpose(
                            pt_all[:, dc * P:(dc + 1) * P],
                            ah[:, dcl * P:(dcl + 1) * P],
                            ident[:],
                        )
                nc.tensor.matmul(
                    po[:], lhsT=ones_sb[:], rhs=bias_sb[:],
                    start=True, stop=False,
                )
                for h in range(2):
                    for dcl in range(NH):
                        dc = h * NH + dcl
                        nc.vector.tensor_copy(
                            out=xt_all[:, dc * P:(dc + 1) * P],
                            in_=pt_all[:, dc * P:(dc + 1) * P],
                        )
                    for dcl in range(NH):
                        dc = h * NH + dcl
                        nc.tensor.matmul(
                            po[:],
                            lhsT=xt_all[:, dc * P:(dc + 1) * P],
                            rhs=wt_sb[:, dc * K:(dc + 1) * K],
                            start=False,
                            stop=(dc == NDC - 1),
                        )
                ot = opool.tile([P, K], F32)
                nc.scalar.activation(
                    ot[:], po[:], mybir.ActivationFunctionType.Tanh
                )
                nc.sync.dma_start(out=yap[NSB - 1, 1], in_=ot[:])

            # Software pipeline per iteration sb:
            #   adds(sb) [DVE, with copies+matmuls+tanh of sb-1 emitted after
            #   the 4th add] -> loads(sb+1) triggers [sync/scalar] ->
            #   transposes(sb) [PE]. No engine's instruction stream ever
            #   waits across the superblock boundary. The last 256 rows run
            #   through the narrow (128-row) pipeline for a short tail.
            xt = issue_loads(0)
            prev = None
            for sb in range(NSB - 1):
                if prev is None:
                    acc = reduce(xt)
                    got = None
                else:
                    psb, ppt = prev
                    got = {}
                    acc = reduce(
                        xt,
                        mid_cb=lambda: got.__setitem__(
                            "ots", project_core(ppt)
                        ),
                    )
                if sb + 1 < NSB - 1:
                    xt = issue_loads(sb + 1)
                else:
                    xtb0 = issue_loads_blk(0)
                    xtb1 = issue_loads_blk(1)
                if got is not None:
                    write_y(psb, got["ots"])
                prev = (sb, transpose_block(acc))
            psb, ppt = prev

            # narrow block A (rows 768..895 of the core shard)
            gotA = {}
            accA = reduce1(
                xtb0,
                mid_cb=lambda: gotA.__setitem__("ots", project_core(ppt)),
            )
            write_y(psb, gotA["ots"])
            ptA = transpose1(accA)

            # narrow block B (rows 896..1023) with the half-split tail
            def tail_mid():
                ot = project1(ptA)
                nc.sync.dma_start(out=yap[NSB - 1, 0], in_=ot[:])

            tail1(xtb1, mid_cb=tail_mid)

    nc.compile()
    _nc_cache = nc
    return nc


def _ensure_ntff_hook():
    """Register the axon NTFF profile hook if the image's antenv lacks it."""
    import sys
    import types

    try:
        from antenv.axon_hooks import get_axon_ntff_profile_hook  # noqa: F401
        return
    except ImportError:
        pass
    import antenv

    mod = types.ModuleType("antenv.axon_hooks")
    mod._hook = None

    def set_axon_ntff_profile_hook(h):
        mod._hook = h

    def get_axon_ntff_profile_hook():
        return mod._hook

    mod.set_axon_ntff_profile_hook = set_axon_ntff_profile_hook
    mod.get_axon_ntff_profile_hook = get_axon_ntff_profile_hook
    sys.modules["antenv.axon_hooks"] = mod
    antenv.axon_hooks = mod
    try:
        from trn_agent_boot.trn_boot import _ntff_profile_via_ctypes

        mod._hook = _ntff_profile_via_ctypes("/opt/axon/libaxon_pjrt.so")
    except Exception:
        mod._hook = None


def _run(inputs, trace=False, **kwargs):
    import ml_dtypes

    bf16 = np.dtype(ml_dtypes.bfloat16)
    x = np.asarray(inputs["x"], dtype=np.float32)
    W = np.asarray(inputs["W"], dtype=np.float32)
    b = np.asarray(inputs["b"], dtype=np.float32)
    wt = (np.ascontiguousarray(W.T) * np.float32(1.0 / L)).astype(bf16)
    bias = np.ascontiguousarray(b.reshape(1, K)).astype(bf16)
    in_maps = [
        {
            "x": np.ascontiguousarray(
                x[:, c * BS:(c + 1) * BS, :]
            ).reshape(L, BS // R, D2),
            "wt": wt,
            "bias": bias,
        }
        for c in range(NCORES)
    ]
    if trace:
        _ensure_ntff_hook()
        import concourse.bass_utils as bu

        bu.upload_artifacts = lambda tmpdir: "local://skipped"
    nc = _build()
    res = run_bass_kernel_spmd(
        nc, in_maps, core_ids=list(range(NCORES)), trace=trace, **kwargs
    )
    # y[sb, j, p, :] is output row sb*256 + 2p + j for sb < NSB-1; the last
    # 256 rows are stored un-interleaved: y[NSB-1, b, p, :] = row 768+128b+p
    def unshuffle(yarr):
        yd = yarr.reshape(NSB, R, P, K)
        top = yd[:NSB - 1].transpose(0, 2, 1, 3).reshape(-1, K)
        bot = yd[NSB - 1].reshape(-1, K)
        return np.concatenate([top, bot], axis=0)

    y = np.concatenate([unshuffle(r["y"]) for r in res.results], axis=0)
    return y, res


def kernel(**inputs):
    y, _ = _run(inputs)
    return y
